# revision 5
# baseline (speedup 1.0000x reference)
"""Trainium2 Bass kernel for nn_AdapterAttnForMamba (depthwise 3x3 conv over a
pad-token-augmented 66x66 image + spatial-transpose permutation + residual).

Math (per batch b, channel c), derived from the reference:
  out(i,j) = x(i,j) + y(j,i) + bias_c
  y(r,s)   = sum_{a,b in 0..2} w[c,a,b] * V[r+a-1, s+b-1]
  V        = 65x65 "virtual" image: V[i<64, j<64] = x(i,j); V[i<64, 64] = tok0;
             V[64, j] = tok_{j%2}; zero outside (SAME conv padding).

Strategy: pure data parallel over batch (2 batches / core, 8 cores).
On-chip layout "A" (channels on partitions) obtained via TensorE transposes;
conv taps are diagonal matmuls accumulating in PSUM (tap shifts are free-dim
AP offsets into a padded SBUF image P). The seq_idx spatial transpose is folded
into the tap access pattern (we read V[j+di, i+dj] directly), so all DMAs stay
in natural, contiguous layout.

Pipeline per (batch bi, channel-block ct of 128):
  x stripes [128pix,1024ch] --(SWDGE cast f32->bf16)--> SBUF
  TensorE transpose -> PSUM [128ch,128pix] -> ACT copy -> P (padded, row stride 80)
  9 diag-matmuls (bf16) accumulate conv into PSUM chunks [128,512]
  ACT evac + conv_b bias -> z[ct] (bf16, l-order = i-major pixels)
  TensorE transpose back -> PSUM [128pix,1024ch]; DVE adds residual x stripe
  SWDGE cast bf16->f32 DMA to DRAM out.
"""

import os
import sys

import numpy as np

for _p in ("/opt/trn_rl_repo", "/root/.axon_site/_ro/trn_rl_repo"):
    if os.path.isdir(_p) and _p not in sys.path:
        sys.path.append(_p)

B, H, W, C = 16, 64, 64, 1024
L = H * W  # 4096
NCORES = 8
BPC = B // NCORES  # batches per core
NCT = C // 128  # channel blocks
RS = 80  # P row stride (elements); >= 67 and 16-aligned (fp8-ready)
PROWS = 66  # P rows: image rows -1..64 (+1 ring)
PSZ = PROWS * RS
NSTRIPE = L // 128  # 32 pixel stripes per batch
TAPS = [(di, dj) for di in (-1, 0, 1) for dj in (-1, 0, 1)]

_CACHE = {}


def _expected_seq_idx():
    return np.arange(L).reshape(H, W).T.reshape(-1)


def _build_nc():
    import concourse.mybir as mybir
    from concourse import bacc
    from concourse.masks import make_identity
    from concourse.tile import TileContext

    f32 = mybir.dt.float32
    bf16 = mybir.dt.bfloat16
    Copy = mybir.ActivationFunctionType.Copy
    Ident = mybir.ActivationFunctionType.Identity

    nc = bacc.Bacc(None, target_bir_lowering=False)
    x_ext = nc.declare_dram_parameter("x", [BPC, L, C], f32, isOutput=False)
    tok_ext = nc.declare_dram_parameter("pad_token", [1, C, 2], f32, isOutput=False)
    w_ext = nc.declare_dram_parameter("conv_w", [C, 1, 3, 3], f32, isOutput=False)
    b_ext = nc.declare_dram_parameter("conv_b", [C], f32, isOutput=False)
    out_ext = nc.declare_dram_parameter("out", [BPC, L, C], f32, isOutput=True)

    with TileContext(nc) as tc:
        with (
            tc.tile_pool(name="const", bufs=1) as constp,
            tc.tile_pool(name="xpool", bufs=34) as xpool,
            tc.tile_pool(name="zpool", bufs=9) as zpool,
            tc.tile_pool(name="ppool", bufs=2) as ppool,
            tc.tile_pool(name="opool", bufs=4) as opool,
            tc.tile_pool(name="ps_t", bufs=2, space="PSUM") as ps_t,
            tc.tile_pool(name="ps_z", bufs=2, space="PSUM") as ps_z,
            tc.tile_pool(name="ps_o", bufs=2, space="PSUM") as ps_o,
        ):
            # ---- constants ----
            ident = constp.tile([128, 128], bf16, tag="ident")
            make_identity(nc, ident)
            zeros = constp.tile([128, 128], bf16, tag="zeros")
            nc.vector.memset(zeros[:], 0.0)

            wt = constp.tile([128, 9 * NCT], f32, tag="wt")
            cb = constp.tile([128, NCT], f32, tag="cb")
            tokt = constp.tile([128, 2 * NCT], f32, tag="tokt")
            nc.sync.dma_start(
                out=wt.rearrange("p (ct t) -> p ct t", t=9),
                in_=w_ext.rearrange("(ct p) a k l -> p ct (a k l)", ct=NCT),
            )
            nc.sync.dma_start(
                out=cb[:],
                in_=b_ext.rearrange("(ct p) -> p ct", ct=NCT),
            )
            nc.sync.dma_start(
                out=tokt.rearrange("p (ct two) -> p ct two", two=2),
                in_=tok_ext.rearrange("a (ct p) two -> p ct (a two)", ct=NCT),
            )

            diag = [[None] * 9 for _ in range(NCT)]
            for ct in range(NCT):
                for t in range(9):
                    d = constp.tile([128, 128], bf16, tag=f"diag{ct}_{t}")
                    nc.vector.tensor_scalar_mul(
                        out=d[:], in0=ident[:], scalar1=wt[:, ct * 9 + t : ct * 9 + t + 1]
                    )
                    diag[ct][t] = d

            # ---- main loops ----
            for bi in range(BPC):
                xs = []
                for s in range(NSTRIPE):
                    xt = xpool.tile([128, C], bf16, tag="x")
                    # SWDGE dma casts f32 -> bf16
                    nc.gpsimd.dma_start(
                        out=xt[:], in_=x_ext[bi, s * 128 : (s + 1) * 128, :]
                    )
                    xs.append(xt)

                z_tiles = []
                for ct in range(NCT):
                    # ---------- phase A: build padded image P (= V + zero ring)
                    P = ppool.tile([128, PSZ], bf16, tag="P")
                    Pv = P.rearrange("p (r c) -> p r c", c=RS)
                    nc.vector.memset(Pv[:, 0:1, 0:66], 0.0)  # ring row (V row -1)
                    nc.vector.memset(Pv[:, 1:66, 0:1], 0.0)  # ring col (V col -1)
                    for g in range(8):
                        pst = ps_t.tile([128, 512], bf16, tag="pst")
                        for s4 in range(4):
                            s = g * 4 + s4
                            nc.tensor.transpose(
                                pst[:, s4 * 128 : (s4 + 1) * 128],
                                xs[s][:, ct * 128 : (ct + 1) * 128],
                                ident[:],
                            )
                        # rows 8g..8g+8 of x -> P rows 8g+1..8g+9, cols 1..65
                        nc.scalar.activation(
                            out=Pv[:, 8 * g + 1 : 8 * g + 9, 1:65],
                            in_=pst[:],
                            func=Copy,
                            scale=1.0,
                        )
                    # pad tokens: V[i,64]=tok0 (i in 0..64), V[64,j]=tok_{j%2}
                    nc.scalar.activation(
                        out=Pv[:, 1:66, 65:66],
                        in_=zeros[:, 0:65],
                        func=Ident,
                        scale=1.0,
                        bias=tokt[:, 2 * ct : 2 * ct + 1],
                    )
                    Pb = P.rearrange("p (r c2 two) -> p r c2 two", c2=RS // 2, two=2)
                    # even j -> P col (j+1) odd = 2*c2+1, c2 in 0..31  (tok0)
                    nc.scalar.activation(
                        out=Pb[:, 65:66, 0:32, 1:2],
                        in_=zeros[:, 0:32],
                        func=Ident,
                        scale=1.0,
                        bias=tokt[:, 2 * ct : 2 * ct + 1],
                    )
                    # odd j -> P col (j+1) even = 2*c2, c2 in 1..32  (tok1)
                    nc.scalar.activation(
                        out=Pb[:, 65:66, 1:33, 0:1],
                        in_=zeros[:, 0:32],
                        func=Ident,
                        scale=1.0,
                        bias=tokt[:, 2 * ct + 1 : 2 * ct + 2],
                    )

                    # ---------- phase B: conv taps via diag matmuls ----------
                    z = zpool.tile([128, L], bf16, tag="z")
                    z_tiles.append(z)
                    # view with column (s-coord) leading: P4[p, c, r]
                    P4 = P.rearrange("p (r c) -> p c r", c=RS)
                    for n in range(8):  # 512-pixel chunks: i rows 8n..8n+8
                        pz = ps_z.tile([128, 512], f32, tag="pz")
                        for t, (di, dj) in enumerate(TAPS):
                            # reads V[j+di, i+dj] for i in chunk rows, all j
                            rhs = P4[
                                :,
                                8 * n + dj + 1 : 8 * n + dj + 9,
                                di + 1 : di + 65,
                            ]
                            nc.tensor.matmul(
                                pz[:],
                                diag[ct][t],
                                rhs,
                                start=(t == 0),
                                stop=(t == 8),
                            )
                        nc.scalar.activation(
                            out=z[:, n * 512 : (n + 1) * 512],
                            in_=pz[:],
                            func=Ident,
                            scale=1.0,
                            bias=cb[:, ct : ct + 1],
                        )

                # ---------- phase C: transpose back, residual, store ----------
                for s in range(NSTRIPE):
                    p2 = ps_o.tile([128, 1024], bf16, tag="p2")
                    for ct in range(NCT):
                        nc.tensor.transpose(
                            p2[:, ct * 128 : (ct + 1) * 128],
                            z_tiles[ct][:, s * 128 : (s + 1) * 128],
                            ident[:],
                        )
                    ob = opool.tile([128, C], bf16, tag="ob")
                    nc.vector.tensor_add(out=ob[:], in0=p2[:], in1=xs[s][:])
                    # SWDGE dma casts bf16 -> f32
                    nc.gpsimd.dma_start(
                        out=out_ext[bi, s * 128 : (s + 1) * 128, :], in_=ob[:]
                    )

    nc.finalize()
    return nc


def _get_compiled():
    if "nc" not in _CACHE:
        _CACHE["nc"] = _build_nc()
    return _CACHE["nc"]


def _run(inputs, trace=False):
    from concourse.bass_utils import run_bass_kernel_spmd

    x = np.ascontiguousarray(np.asarray(inputs["x"], dtype=np.float32))
    pad_token = np.ascontiguousarray(np.asarray(inputs["pad_token"], dtype=np.float32))
    conv_w = np.ascontiguousarray(np.asarray(inputs["conv_w"], dtype=np.float32))
    conv_b = np.ascontiguousarray(np.asarray(inputs["conv_b"], dtype=np.float32))
    seq_idx = np.asarray(inputs["seq_idx"]).astype(np.int64)

    nc = _get_compiled()
    in_maps = []
    for k in range(NCORES):
        in_maps.append(
            {
                "x": x[k * BPC : (k + 1) * BPC],
                "pad_token": pad_token,
                "conv_w": conv_w,
                "conv_b": conv_b,
            }
        )
    res = run_bass_kernel_spmd(nc, in_maps, core_ids=list(range(NCORES)), trace=trace)
    out = np.concatenate([r["out"] for r in res.results], axis=0)

    # The device kernel hardcodes the reference's transpose permutation in its
    # access patterns. If the harness ever supplies a different seq_idx,
    # correct on host: out = x + y[:, seq_idx]  with y = (out_dev - x) at the
    # hardcoded permutation undone.
    exp = _expected_seq_idx()
    if not np.array_equal(seq_idx, exp):
        y = (out - x)[:, np.argsort(exp), :]
        out = x + y[:, seq_idx, :]

    return out, getattr(res, "exec_time_ns", None)


def kernel(**inputs) -> np.ndarray:
    out, _ = _run(inputs, trace=False)
    return out
